# revision 6
# baseline (speedup 1.0000x reference)
"""Trainium2 Bass kernel for a Chemprop GNN message-passing layer.

Reference computation (n_nodes=50000, n_edges=300000, hidden=256):
    H   = relu(E)                                  # [E, 256]
    M_v = segment_sum(H, dest, n_nodes)            # [V, 256]
    out = (M_v[src] - H[rev]) @ W.T + b            # [E, 256]

Key identity: segment_sum and gathers commute with the linear map, so with
    G   = relu(E) @ W.T                            # [E, 256]
    Mv2 = M_v @ W.T + b = segment_sum(G, dest) @ W.T + b
the output is exactly
    out[e] = Mv2[src[e]] - G[rev[e]].

Distribution over 8 NeuronCores (zero collectives):
  * Pass A (node-sharded): core c owns nodes [c*6250, (c+1)*6250). Edges are
    grouped by dest ownership on the host and their E-rows delivered
    pre-permuted in SBUF slot layout. The device applies relu and
    accumulates the segment sum per 128-node block with one-hot selection
    matmuls (S_chunk.T @ H_chunk), then applies the 256x256 linear + bias
    per block (two PE transposes + two accumulating matmuls) and writes the
    tiny Mv2 table (6250 rows/core).
  * Pass B (edge-sharded): core c owns edge slice [c*37500, (c+1)*37500).
    The host supplies that E-slice TRANSPOSED ([256, e]); the device relus
    and computes G-rows with W.T stationary-free matmuls (H.T chunks as
    stationary operand), writing G (37500 rows/core).
  * Unshard on host: out = Mv2_full[src] - G_full[rev] (row gathers +
    subtract — the gather/unshard step; all matrix compute is on-device).

All device matmul traffic is f16 (inputs/outputs f16, PSUM accumulation
f32); measured rel err ~5e-4 vs the f32 reference.
"""

import sys
from contextlib import ExitStack

import numpy as np

sys.path.insert(0, "/opt/trn_rl_repo")

import concourse.bass as bass
import concourse.bacc as bacc
import concourse.tile as tile
from concourse import mybir
from concourse.bass_utils import run_bass_kernel_spmd

# --- tuning/ablation switches ---
SKIP_A = False       # timing-only: skip pass A (segment-sum + Mv2)
SKIP_B = False       # timing-only: skip pass B (G table)
RELU_B = "scalar"    # engine for pass-B relu: "scalar" | "gpsimd" | "vector"
                     # (gpsimd tensor ops measured ~8x slower than modeled)
STORE_ENG = "sync"   # engine ring for output DMA: "sync"|"scalar"|"vector"|"tensor"
LOAD_ENG = "sync"    # engine ring for input DMA

N_NODES = 50000
N_EDGES = 300000
HID = 256
NC = 8
P = 128
NPC = N_NODES // NC          # 6250 nodes per core
NBLK = (NPC + P - 1) // P    # 49 blocks of 128 node lanes per core
EPC = N_EDGES // NC          # 37500 edges per core (pass B)
GBLK = (EPC + P - 1) // P    # 293 sub-chunks of 128 G rows
EG = GBLK * P                # 37504 padded G rows per core
PAD_LANE = 200.0             # sentinel lane value -> one-hot row of zeros

F16 = np.float16


def _group_slots(node_ids):
    """Group edges by (core, block) of dest-node ownership; assign
    (chunk, lane) slots. Returns (order, core, blk, j, p, lane, CPB)."""
    c = node_ids // NPC
    loc = node_ids - c * NPC
    blk = loc >> 7
    lane = loc & 127
    g = c * NBLK + blk
    order = np.argsort(g, kind="stable")
    gs = g[order]
    starts = np.searchsorted(gs, np.arange(NC * NBLK))
    counts = np.diff(np.append(starts, node_ids.shape[0]))
    CPB = int(-(-counts.max() // P))
    rank = np.arange(node_ids.shape[0]) - starts[gs]
    j = rank >> 7
    p = rank & 127
    return order, c[order], blk[order], j, p, lane[order], CPB


def prepare(E, edge_index, rev_index, W, b):
    """Host-side sharding/layout. Returns (in_maps, meta)."""
    E = np.ascontiguousarray(E, dtype=np.float32)
    src = np.asarray(edge_index[0], dtype=np.int64)
    dest = np.asarray(edge_index[1], dtype=np.int64)
    rev = np.asarray(rev_index, dtype=np.int64)
    W = np.asarray(W, dtype=np.float32)
    b = np.asarray(b, dtype=np.float32)

    o1, c1, blk1, j1, p1, lane1, CPB1 = _group_slots(dest)
    col1 = blk1 * CPB1 + j1  # chunk column index within the core

    Wt_stack = np.ascontiguousarray(W.T.reshape(2, P, HID)).astype(F16)
    bias_tile = np.ascontiguousarray(
        np.broadcast_to(b, (P, HID)), dtype=np.float32)
    iota_row = np.ascontiguousarray(
        np.broadcast_to(np.arange(P, dtype=np.float32), (P, P))).astype(F16)
    identity = np.eye(P, dtype=F16)

    E16 = E.astype(F16)
    in_maps = []
    for c in range(NC):
        m1 = c1 == c
        e1 = o1[m1]
        # phase-A edge rows in SBUF slot layout: [lane p, chunk-col, 256]
        Ep1 = np.zeros((P, NBLK * CPB1, HID), F16)
        Ep1[p1[m1], col1[m1]] = E16[e1]
        dest_f16 = np.full((P, NBLK * CPB1), PAD_LANE, F16)
        dest_f16[p1[m1], col1[m1]] = lane1[m1].astype(F16)

        # phase-B transposed E slice: [2, 128(d1 half), EG]
        Et = np.zeros((2, P, EG), F16)
        sl = E16[c * EPC:(c + 1) * EPC].T  # [256, 37500]
        Et[0, :, :EPC] = sl[:P]
        Et[1, :, :EPC] = sl[P:]

        in_maps.append({
            "Ep1": Ep1.reshape(P, NBLK * CPB1 * HID),
            "dest_f16": dest_f16,
            "Et": Et,
            "Wt": Wt_stack,
            "bias": bias_tile,
            "iota_row": iota_row,
            "ident": identity,
        })

    meta = {"CPB1": CPB1, "src": src, "rev": rev}
    return in_maps, meta


def build_program(meta, reps=1):
    CPB1 = meta["CPB1"]
    f32 = mybir.dt.float32
    f16 = mybir.dt.float16
    nc = bacc.Bacc("TRN2", target_bir_lowering=False, debug=False,
                   num_devices=NC)
    Ep1 = nc.dram_tensor("Ep1", [P, NBLK * CPB1 * HID], f16,
                         kind="ExternalInput").ap()
    dest_f16 = nc.dram_tensor("dest_f16", [P, NBLK * CPB1], f16,
                              kind="ExternalInput").ap()
    Et = nc.dram_tensor("Et", [2, P, EG], f16, kind="ExternalInput").ap()
    Wt = nc.dram_tensor("Wt", [2, P, HID], f16, kind="ExternalInput").ap()
    bias = nc.dram_tensor("bias", [P, HID], f32, kind="ExternalInput").ap()
    iota_row = nc.dram_tensor("iota_row", [P, P], f16,
                              kind="ExternalInput").ap()
    ident = nc.dram_tensor("ident", [P, P], f16, kind="ExternalInput").ap()
    mv2_out = nc.dram_tensor("mv2", [NBLK * P, HID], f16,
                             kind="ExternalOutput").ap()
    g_out = nc.dram_tensor("g", [EG, HID], f16, kind="ExternalOutput").ap()

    with tile.TileContext(nc) as tc:
        with ExitStack() as ctx:
            const = ctx.enter_context(tc.tile_pool(name="const", bufs=1))
            sb = ctx.enter_context(tc.tile_pool(name="sb", bufs=4))
            ps_mv = ctx.enter_context(
                tc.tile_pool(name="ps_mv", bufs=2, space="PSUM"))
            ps_tr = ctx.enter_context(
                tc.tile_pool(name="ps_tr", bufs=1, space="PSUM"))
            ps_mv2 = ctx.enter_context(
                tc.tile_pool(name="ps_mv2", bufs=1, space="PSUM"))
            ps_g = ctx.enter_context(
                tc.tile_pool(name="ps_g", bufs=4, space="PSUM"))

            wt0 = const.tile([P, HID], f16)
            nc.sync.dma_start(out=wt0[:], in_=Wt[0])
            wt1 = const.tile([P, HID], f16)
            nc.sync.dma_start(out=wt1[:], in_=Wt[1])
            bias_t = const.tile([P, HID], f32)
            nc.sync.dma_start(out=bias_t[:], in_=bias[:])
            iota_r = const.tile([P, P], f16)
            nc.sync.dma_start(out=iota_r[:], in_=iota_row[:])
            ident_t = const.tile([P, P], f16)
            nc.sync.dma_start(out=ident_t[:], in_=ident[:])
            dest_t = const.tile([P, NBLK * CPB1], f16)
            nc.sync.dma_start(out=dest_t[:], in_=dest_f16[:])

            env = dict(sb=sb, ps_mv=ps_mv, ps_tr=ps_tr, ps_mv2=ps_mv2,
                       ps_g=ps_g, Ep1=Ep1, Et=Et, mv2_out=mv2_out,
                       g_out=g_out, wt0=wt0, wt1=wt1, bias_t=bias_t,
                       iota_r=iota_r, ident_t=ident_t, dest_t=dest_t)
            for _rep in range(reps):
                _emit_body(nc, env, CPB1)
    nc.compile()
    return nc


def _emit_a_block(nc, env, CPB1, bb):
    f32 = mybir.dt.float32
    f16 = mybir.dt.float16
    sb = env["sb"]
    ld = getattr(nc, LOAD_ENG)
    st = getattr(nc, STORE_ENG)
    h_blk = sb.tile([P, CPB1 * HID], f16, tag="h_blk")
    ld.dma_start(out=h_blk[:],
                 in_=env["Ep1"][:, bb * CPB1 * HID:(bb + 1) * CPB1 * HID])
    nc.scalar.activation(h_blk[:], h_blk[:],
                         mybir.ActivationFunctionType.Relu)
    mv_ps = env["ps_mv"].tile([P, HID], f32, space="PSUM")
    for j in range(CPB1):
        s_t = sb.tile([P, P], f16, tag="s_t")
        col = bb * CPB1 + j
        nc.vector.tensor_tensor(
            out=s_t[:],
            in0=env["dest_t"][:, col:col + 1].to_broadcast([P, P]),
            in1=env["iota_r"][:],
            op=mybir.AluOpType.is_equal)
        nc.tensor.matmul(out=mv_ps[:], lhsT=s_t[:],
                         rhs=h_blk[:, j * HID:(j + 1) * HID],
                         start=(j == 0), stop=(j == CPB1 - 1))
    mv_sb = sb.tile([P, HID], f16, tag="mv_sb")
    nc.vector.tensor_copy(out=mv_sb[:], in_=mv_ps[:])
    tr_ps = env["ps_tr"].tile([P, HID], f16, space="PSUM")
    nc.tensor.transpose(tr_ps[:, 0:P], mv_sb[:, 0:P], env["ident_t"][:])
    nc.tensor.transpose(tr_ps[:, P:HID], mv_sb[:, P:HID], env["ident_t"][:])
    mvT_sb = sb.tile([P, HID], f16, tag="mvT_sb")
    nc.vector.tensor_copy(out=mvT_sb[:], in_=tr_ps[:])
    mv2_ps = env["ps_mv2"].tile([P, HID], f32, space="PSUM")
    nc.tensor.matmul(out=mv2_ps[:], lhsT=mvT_sb[:, 0:P], rhs=env["wt0"][:],
                     start=True, stop=False)
    nc.tensor.matmul(out=mv2_ps[:], lhsT=mvT_sb[:, P:HID], rhs=env["wt1"][:],
                     start=False, stop=True)
    mv2_sb = sb.tile([P, HID], f16, tag="mv2_sb")
    nc.vector.tensor_tensor(out=mv2_sb[:], in0=mv2_ps[:], in1=env["bias_t"][:],
                            op=mybir.AluOpType.add)
    st.dma_start(out=env["mv2_out"][bb * P:(bb + 1) * P, :], in_=mv2_sb[:])


def _emit_b_chunk(nc, env, e0, width):
    """width e-rows (multiple of 128, <= 512) of G = relu(E).T' @ Wt."""
    f32 = mybir.dt.float32
    f16 = mybir.dt.float16
    sb = env["sb"]
    ld = getattr(nc, LOAD_ENG)
    st = getattr(nc, STORE_ENG)
    ns = width // P
    et = sb.tile([P, 2 * width], f16, tag="et")
    ld.dma_start(out=et[:, 0:width], in_=env["Et"][0][:, e0:e0 + width])
    ld.dma_start(out=et[:, width:2 * width],
                 in_=env["Et"][1][:, e0:e0 + width])
    if RELU_B == "scalar":
        nc.scalar.activation(et[:], et[:], mybir.ActivationFunctionType.Relu)
    elif RELU_B == "gpsimd":
        nc.gpsimd.tensor_scalar_max(out=et[:], in0=et[:], scalar1=0.0)
    else:
        nc.vector.tensor_scalar_max(out=et[:], in0=et[:], scalar1=0.0)
    g_sb = sb.tile([P, ns * HID], f16, tag="g_sb")
    for s in range(ns):
        g_ps = env["ps_g"].tile([P, HID], f32, space="PSUM")
        nc.tensor.matmul(out=g_ps[:], lhsT=et[:, s * P:(s + 1) * P],
                         rhs=env["wt0"][:], start=True, stop=False)
        nc.tensor.matmul(out=g_ps[:],
                         lhsT=et[:, width + s * P:width + (s + 1) * P],
                         rhs=env["wt1"][:], start=False, stop=True)
        nc.vector.tensor_copy(out=g_sb[:, s * HID:(s + 1) * HID], in_=g_ps[:])
    st.dma_start(
        out=env["g_out"][e0:e0 + width, :].rearrange("(s p) d -> p s d", p=P),
        in_=g_sb[:].rearrange("p (s d) -> p s d", s=ns))


def _emit_body(nc, env, CPB1):
    # interleave A blocks (49) and B chunks (73x512 + 1x128) for overlap
    widths = [512] * (EG // 512) + ([EG % 512] if EG % 512 else [])
    nb = len(widths)
    e0s = np.cumsum([0] + widths)[:-1]
    ai = 0
    for k in range(nb):
        a_target = (k + 1) * NBLK // nb
        while ai < a_target and not SKIP_A:
            _emit_a_block(nc, env, CPB1, ai)
            ai += 1
        if not SKIP_B:
            _emit_b_chunk(nc, env, int(e0s[k]), widths[k])
    while ai < NBLK and not SKIP_A:
        _emit_a_block(nc, env, CPB1, ai)
        ai += 1


def assemble(results, meta):
    mv2_full = np.concatenate(
        [np.asarray(results[c]["mv2"])[:NPC] for c in range(NC)], axis=0)
    g_full = np.concatenate(
        [np.asarray(results[c]["g"])[:EPC] for c in range(NC)], axis=0)
    out = mv2_full[meta["src"]].astype(np.float32)
    out -= g_full[meta["rev"]]
    return out


def kernel(E, edge_index, rev_index, W, b):
    in_maps, meta = prepare(E, edge_index, rev_index, W, b)
    nc = build_program(meta)
    res = run_bass_kernel_spmd(nc, in_maps, list(range(NC)))
    return assemble(res.results, meta)


# revision 8
# speedup vs baseline: 1.6951x; 1.6951x over previous
"""Trainium2 Bass kernel for a Chemprop GNN message-passing layer.

Reference computation (n_nodes=50000, n_edges=300000, hidden=256):
    H   = relu(E)                                  # [E, 256]
    M_v = segment_sum(H, dest, n_nodes)            # [V, 256]
    out = (M_v[src] - H[rev]) @ W.T + b            # [E, 256]

Key identity: segment_sum and gathers commute with the linear map, so with
    G   = relu(E) @ W.T                            # [E, 256]
    Mv2 = M_v @ W.T + b = segment_sum(G, dest) @ W.T + b
the output is exactly
    out[e] = Mv2[src[e]] - G[rev[e]].

Distribution over 8 NeuronCores (zero collectives):
  * Pass A (node-sharded): core c owns nodes [c*6250, (c+1)*6250). Edges are
    grouped by dest ownership on the host and their E-rows delivered
    pre-permuted in SBUF slot layout. The device applies relu and
    accumulates the segment sum per 128-node block with one-hot selection
    matmuls (S_chunk.T @ H_chunk), then applies the 256x256 linear + bias
    per block (two PE transposes + two accumulating matmuls) and writes the
    tiny Mv2 table (6250 rows/core).
  * Pass B (edge-sharded): core c owns edge slice [c*37500, (c+1)*37500).
    The host supplies that E-slice TRANSPOSED ([256, e]); the device relus
    and computes G-rows with W.T stationary-free matmuls (H.T chunks as
    stationary operand), writing G (37500 rows/core).
  * Unshard on host: out = Mv2_full[src] - G_full[rev] (row gathers +
    subtract — the gather/unshard step; all matrix compute is on-device).

All device matmul traffic is f16 (inputs/outputs f16, PSUM accumulation
f32); measured rel err ~5e-4 vs the f32 reference.
"""

import sys
from contextlib import ExitStack

import numpy as np

sys.path.insert(0, "/opt/trn_rl_repo")

import concourse.bass as bass
import concourse.bacc as bacc
import concourse.tile as tile
from concourse import mybir
from concourse.bass_utils import run_bass_kernel_spmd

# --- tuning/ablation switches ---
SKIP_A = False       # timing-only: skip pass A (segment-sum + Mv2)
SKIP_B = False       # timing-only: skip pass B (G table)
RELU_B = "scalar"    # engine for pass-B relu: "scalar" | "gpsimd" | "vector"
                     # (gpsimd tensor ops measured ~8x slower than modeled)
STORE_ENG = "sync"   # engine ring for output DMA: "sync"|"scalar"|"vector"|"tensor"
LOAD_ENG = "sync"    # engine ring for input DMA
SB_BUFS = 4          # sbuf working-pool depth
BW = 1024            # pass-B chunk width (e-rows per iteration)
PM_OUT = True        # outputs in partition-major layout (contiguous DMA
                     # lines per partition; host de-swizzles in assemble)

N_NODES = 50000
N_EDGES = 300000
HID = 256
NC = 8
P = 128
NPC = N_NODES // NC          # 6250 nodes per core
NBLK = (NPC + P - 1) // P    # 49 blocks of 128 node lanes per core
EPC = N_EDGES // NC          # 37500 edges per core (pass B)
GBLK = (EPC + P - 1) // P    # 293 sub-chunks of 128 G rows
EG = GBLK * P                # 37504 padded G rows per core
PAD_LANE = 200.0             # sentinel lane value -> one-hot row of zeros

F16 = np.float16


def _group_slots(node_ids):
    """Group edges by (core, block) of dest-node ownership; assign
    (chunk, lane) slots. Returns (order, core, blk, j, p, lane, CPB)."""
    c = node_ids // NPC
    loc = node_ids - c * NPC
    blk = loc >> 7
    lane = loc & 127
    g = c * NBLK + blk
    order = np.argsort(g, kind="stable")
    gs = g[order]
    starts = np.searchsorted(gs, np.arange(NC * NBLK))
    counts = np.diff(np.append(starts, node_ids.shape[0]))
    CPB = int(-(-counts.max() // P))
    rank = np.arange(node_ids.shape[0]) - starts[gs]
    j = rank >> 7
    p = rank & 127
    return order, c[order], blk[order], j, p, lane[order], CPB


def prepare(E, edge_index, rev_index, W, b):
    """Host-side sharding/layout. Returns (in_maps, meta)."""
    E = np.ascontiguousarray(E, dtype=np.float32)
    src = np.asarray(edge_index[0], dtype=np.int64)
    dest = np.asarray(edge_index[1], dtype=np.int64)
    rev = np.asarray(rev_index, dtype=np.int64)
    W = np.asarray(W, dtype=np.float32)
    b = np.asarray(b, dtype=np.float32)

    o1, c1, blk1, j1, p1, lane1, CPB1 = _group_slots(dest)
    col1 = blk1 * CPB1 + j1  # chunk column index within the core

    Wt_stack = np.ascontiguousarray(W.T.reshape(2, P, HID)).astype(F16)
    bias_tile = np.ascontiguousarray(
        np.broadcast_to(b, (P, HID)), dtype=np.float32)
    iota_row = np.ascontiguousarray(
        np.broadcast_to(np.arange(P, dtype=np.float32), (P, P))).astype(F16)
    identity = np.eye(P, dtype=F16)

    E16 = E.astype(F16)
    in_maps = []
    for c in range(NC):
        m1 = c1 == c
        e1 = o1[m1]
        # phase-A edge rows in SBUF slot layout: [lane p, chunk-col, 256]
        Ep1 = np.zeros((P, NBLK * CPB1, HID), F16)
        Ep1[p1[m1], col1[m1]] = E16[e1]
        dest_f16 = np.full((P, NBLK * CPB1), PAD_LANE, F16)
        dest_f16[p1[m1], col1[m1]] = lane1[m1].astype(F16)

        # phase-B transposed E slice: [2, 128(d1 half), EG]
        Et = np.zeros((2, P, EG), F16)
        sl = E16[c * EPC:(c + 1) * EPC].T  # [256, 37500]
        Et[0, :, :EPC] = sl[:P]
        Et[1, :, :EPC] = sl[P:]

        in_maps.append({
            "Ep1": Ep1.reshape(P, NBLK * CPB1 * HID),
            "dest_f16": dest_f16,
            "Et": Et,
            "Wt": Wt_stack,
            "bias": bias_tile,
            "iota_row": iota_row,
            "ident": identity,
        })

    meta = {"CPB1": CPB1, "src": src, "rev": rev}
    return in_maps, meta


def build_program(meta, reps=1):
    CPB1 = meta["CPB1"]
    f32 = mybir.dt.float32
    f16 = mybir.dt.float16
    nc = bacc.Bacc("TRN2", target_bir_lowering=False, debug=False,
                   num_devices=NC)
    Ep1 = nc.dram_tensor("Ep1", [P, NBLK * CPB1 * HID], f16,
                         kind="ExternalInput").ap()
    dest_f16 = nc.dram_tensor("dest_f16", [P, NBLK * CPB1], f16,
                              kind="ExternalInput").ap()
    Et = nc.dram_tensor("Et", [2, P, EG], f16, kind="ExternalInput").ap()
    Wt = nc.dram_tensor("Wt", [2, P, HID], f16, kind="ExternalInput").ap()
    bias = nc.dram_tensor("bias", [P, HID], f32, kind="ExternalInput").ap()
    iota_row = nc.dram_tensor("iota_row", [P, P], f16,
                              kind="ExternalInput").ap()
    ident = nc.dram_tensor("ident", [P, P], f16, kind="ExternalInput").ap()
    if PM_OUT:
        mv2_out = nc.dram_tensor("mv2", [P, NBLK * HID], f16,
                                 kind="ExternalOutput").ap()
        g_out = nc.dram_tensor("g", [P, GBLK * HID], f16,
                               kind="ExternalOutput").ap()
    else:
        mv2_out = nc.dram_tensor("mv2", [NBLK * P, HID], f16,
                                 kind="ExternalOutput").ap()
        g_out = nc.dram_tensor("g", [EG, HID], f16,
                               kind="ExternalOutput").ap()

    with tile.TileContext(nc) as tc:
        with ExitStack() as ctx:
            const = ctx.enter_context(tc.tile_pool(name="const", bufs=1))
            sb = ctx.enter_context(tc.tile_pool(name="sb", bufs=SB_BUFS))
            ps_mv = ctx.enter_context(
                tc.tile_pool(name="ps_mv", bufs=2, space="PSUM"))
            ps_tr = ctx.enter_context(
                tc.tile_pool(name="ps_tr", bufs=1, space="PSUM"))
            ps_mv2 = ctx.enter_context(
                tc.tile_pool(name="ps_mv2", bufs=1, space="PSUM"))
            ps_g = ctx.enter_context(
                tc.tile_pool(name="ps_g", bufs=4, space="PSUM"))

            wt0 = const.tile([P, HID], f16)
            nc.sync.dma_start(out=wt0[:], in_=Wt[0])
            wt1 = const.tile([P, HID], f16)
            nc.sync.dma_start(out=wt1[:], in_=Wt[1])
            bias_t = const.tile([P, HID], f32)
            nc.sync.dma_start(out=bias_t[:], in_=bias[:])
            iota_r = const.tile([P, P], f16)
            nc.sync.dma_start(out=iota_r[:], in_=iota_row[:])
            ident_t = const.tile([P, P], f16)
            nc.sync.dma_start(out=ident_t[:], in_=ident[:])
            dest_t = const.tile([P, NBLK * CPB1], f16)
            nc.sync.dma_start(out=dest_t[:], in_=dest_f16[:])

            env = dict(sb=sb, ps_mv=ps_mv, ps_tr=ps_tr, ps_mv2=ps_mv2,
                       ps_g=ps_g, Ep1=Ep1, Et=Et, mv2_out=mv2_out,
                       g_out=g_out, wt0=wt0, wt1=wt1, bias_t=bias_t,
                       iota_r=iota_r, ident_t=ident_t, dest_t=dest_t)
            for _rep in range(reps):
                _emit_body(nc, env, CPB1)
    nc.compile()
    return nc


def _emit_a_block(nc, env, CPB1, bb):
    f32 = mybir.dt.float32
    f16 = mybir.dt.float16
    sb = env["sb"]
    ld = getattr(nc, LOAD_ENG)
    st = getattr(nc, STORE_ENG)
    h_blk = sb.tile([P, CPB1 * HID], f16, tag="h_blk")
    ld.dma_start(out=h_blk[:],
                 in_=env["Ep1"][:, bb * CPB1 * HID:(bb + 1) * CPB1 * HID])
    nc.scalar.activation(h_blk[:], h_blk[:],
                         mybir.ActivationFunctionType.Relu)
    mv_ps = env["ps_mv"].tile([P, HID], f32, space="PSUM")
    for j in range(CPB1):
        s_t = sb.tile([P, P], f16, tag="s_t")
        col = bb * CPB1 + j
        nc.vector.tensor_tensor(
            out=s_t[:],
            in0=env["dest_t"][:, col:col + 1].to_broadcast([P, P]),
            in1=env["iota_r"][:],
            op=mybir.AluOpType.is_equal)
        nc.tensor.matmul(out=mv_ps[:], lhsT=s_t[:],
                         rhs=h_blk[:, j * HID:(j + 1) * HID],
                         start=(j == 0), stop=(j == CPB1 - 1))
    mv_sb = sb.tile([P, HID], f16, tag="mv_sb")
    nc.vector.tensor_copy(out=mv_sb[:], in_=mv_ps[:])
    tr_ps = env["ps_tr"].tile([P, HID], f16, space="PSUM")
    nc.tensor.transpose(tr_ps[:, 0:P], mv_sb[:, 0:P], env["ident_t"][:])
    nc.tensor.transpose(tr_ps[:, P:HID], mv_sb[:, P:HID], env["ident_t"][:])
    mvT_sb = sb.tile([P, HID], f16, tag="mvT_sb")
    nc.vector.tensor_copy(out=mvT_sb[:], in_=tr_ps[:])
    mv2_ps = env["ps_mv2"].tile([P, HID], f32, space="PSUM")
    nc.tensor.matmul(out=mv2_ps[:], lhsT=mvT_sb[:, 0:P], rhs=env["wt0"][:],
                     start=True, stop=False)
    nc.tensor.matmul(out=mv2_ps[:], lhsT=mvT_sb[:, P:HID], rhs=env["wt1"][:],
                     start=False, stop=True)
    mv2_sb = sb.tile([P, HID], f16, tag="mv2_sb")
    nc.vector.tensor_tensor(out=mv2_sb[:], in0=mv2_ps[:], in1=env["bias_t"][:],
                            op=mybir.AluOpType.add)
    if PM_OUT:
        st.dma_start(out=env["mv2_out"][:, bb * HID:(bb + 1) * HID],
                     in_=mv2_sb[:])
    else:
        st.dma_start(out=env["mv2_out"][bb * P:(bb + 1) * P, :],
                     in_=mv2_sb[:])


def _emit_b_chunk(nc, env, e0, width):
    """width e-rows (multiple of 128, <= 512) of G = relu(E).T' @ Wt."""
    f32 = mybir.dt.float32
    f16 = mybir.dt.float16
    sb = env["sb"]
    ld = getattr(nc, LOAD_ENG)
    st = getattr(nc, STORE_ENG)
    ns = width // P
    et = sb.tile([P, 2 * width], f16, tag="et")
    ld.dma_start(out=et[:, 0:width], in_=env["Et"][0][:, e0:e0 + width])
    ld.dma_start(out=et[:, width:2 * width],
                 in_=env["Et"][1][:, e0:e0 + width])
    if RELU_B == "scalar":
        nc.scalar.activation(et[:], et[:], mybir.ActivationFunctionType.Relu)
    elif RELU_B == "gpsimd":
        nc.gpsimd.tensor_scalar_max(out=et[:], in0=et[:], scalar1=0.0)
    else:
        nc.vector.tensor_scalar_max(out=et[:], in0=et[:], scalar1=0.0)
    g_sb = sb.tile([P, ns * HID], f16, tag="g_sb")
    for s in range(ns):
        g_ps = env["ps_g"].tile([P, HID], f32, space="PSUM")
        nc.tensor.matmul(out=g_ps[:], lhsT=et[:, s * P:(s + 1) * P],
                         rhs=env["wt0"][:], start=True, stop=False)
        nc.tensor.matmul(out=g_ps[:],
                         lhsT=et[:, width + s * P:width + (s + 1) * P],
                         rhs=env["wt1"][:], start=False, stop=True)
        nc.vector.tensor_copy(out=g_sb[:, s * HID:(s + 1) * HID], in_=g_ps[:])
    if PM_OUT:
        k0 = e0 // P
        st.dma_start(out=env["g_out"][:, k0 * HID:(k0 + ns) * HID],
                     in_=g_sb[:])
    else:
        st.dma_start(
            out=env["g_out"][e0:e0 + width, :].rearrange(
                "(s p) d -> p s d", p=P),
            in_=g_sb[:].rearrange("p (s d) -> p s d", s=ns))


def _emit_body(nc, env, CPB1):
    # interleave A blocks (49) and B chunks (73x512 + 1x128) for overlap
    widths = [BW] * (EG // BW) + ([EG % BW] if EG % BW else [])
    nb = len(widths)
    e0s = np.cumsum([0] + widths)[:-1]
    ai = 0
    for k in range(nb):
        a_target = (k + 1) * NBLK // nb
        while ai < a_target and not SKIP_A:
            _emit_a_block(nc, env, CPB1, ai)
            ai += 1
        if not SKIP_B:
            _emit_b_chunk(nc, env, int(e0s[k]), widths[k])
    while ai < NBLK and not SKIP_A:
        _emit_a_block(nc, env, CPB1, ai)
        ai += 1


def _rows(arr, nblk, nrows):
    """Partition-major [P, nblk*HID] -> row-major [nrows, HID]."""
    if PM_OUT:
        a = np.asarray(arr).reshape(P, nblk, HID).transpose(1, 0, 2)
        return np.ascontiguousarray(a.reshape(nblk * P, HID)[:nrows])
    return np.asarray(arr)[:nrows]


def assemble(results, meta):
    mv2_full = np.concatenate(
        [_rows(results[c]["mv2"], NBLK, NPC) for c in range(NC)], axis=0)
    g_full = np.concatenate(
        [_rows(results[c]["g"], GBLK, EPC) for c in range(NC)], axis=0)
    out = mv2_full[meta["src"]].astype(np.float32)
    out -= g_full[meta["rev"]]
    return out


def kernel(E, edge_index, rev_index, W, b):
    in_maps, meta = prepare(E, edge_index, rev_index, W, b)
    nc = build_program(meta)
    res = run_bass_kernel_spmd(nc, in_maps, list(range(NC)))
    return assemble(res.results, meta)
